# revision 28
# baseline (speedup 1.0000x reference)
"""Trainium2 Bass kernel for nn_AttentionSE3 (graph attention message passing).

Strategy (edge/graph parallel, fully host-prepped ELL layout):
- Attention is a segment softmax over incoming edges of each dst node.  Logits are
  dot(k_edge, q_dst)/sqrt(128) with k,q ~ N(0,1): |logit| <~ 2, so the max-subtraction
  is dropped (softmax is shift-invariant; exp() never overflows here) and
  out[n] = sum_e exp(logit_e) * v_e / sum_e exp(logit_e).
- Host sorts nodes by in-degree, packs them into 128-node blocks, and pads each
  block's per-node edge lists to the block max degree D (rounded to a multiple
  of 4; degree sorting keeps padding small).  Blocks are dealt round-robin to
  the 8 cores; per-group capacity is the max over the 8 cores so EVERY core
  runs the same static program (no collectives: no node's edges span cores).
- Host packs ONE interleaved kv array: per (group, node-row) the D key rows
  [D,128] then the D value rows [D,96] (value columns permuted to [c12,h8]).
  One DMA per group loads both.
- Device program per group-run (consecutive equal-D groups merge into one set
  of wide ops), software-pipelined so no engine's in-order queue waits on
  same-iteration cross-engine results:
  iteration i emits:  k*q multiply(i) (q broadcast over d; a SMALL slice on
  GPSIMD -- heavy concurrent GPSIMD work poisons VectorE's SBUF perf modes);
  VectorE tensor_reduce over contiguous inner k=16 -> logits(i); ScalarE
  exp(i) and the c-expansion ewx(i); then the DEFERRED value side of run i-1
  (multiply, two contiguous d-halvings, strided reduce); then the
  denominator reduce(i).  Padded slots contribute exp(0)=1; a per-node pad
  count is subtracted (exact).  Output accumulates in SBUF; one wide
  normalization pass + one store at the end.
"""

import numpy as np

import concourse.bacc as bacc
import concourse.mybir as mybir
from concourse import tile
from concourse.bass_utils import run_bass_kernel_spmd

try:
    import ml_dtypes
    BF16_NP = np.dtype(ml_dtypes.bfloat16)
except ImportError:  # pragma: no cover
    BF16_NP = None

N_NODES = 50000
H = 8
P = 128  # nodes per block
N_CORES = 8
SCALE = float(1.0 / np.sqrt(128.0))
F32 = mybir.dt.float32
DT_NP = BF16_NP

# engine split knobs (fraction to GPSIMD; VectorE takes the rest).  Keep these
# SMALL: concurrent GPSIMD traffic degrades VectorE's SBUF perf modes.
GP_FRAC_KMUL = 0.22   # of the k*q multiply d-range
GP_FRAC_VMUL = 0.12   # of the v*ewx multiply d-range
D_ROUND = 4           # capacities rounded up to this multiple (2 halvings)
MERGE_CAP = 48        # max merged capacity per run (SBUF-bounded)

# value columns permuted from [h(8), cx(12)] to [cx(12), h(8)] to match the
# ewx expansion layout (c outer, h inner)
PERM_V = np.arange(96).reshape(8, 12).T.reshape(-1)  # new cx*8+h -> old h*12+cx
PERM_V_INV = np.argsort(PERM_V)


# ---------------------------------------------------------------- host prep

def prepare(value, key, query0, query1, edge_index, n_nodes=N_NODES, n_cores=N_CORES):
    """Build per-core padded ELL shards.  Returns (in_maps, meta)."""
    value = np.asarray(value, dtype=np.float32)
    key = np.asarray(key, dtype=np.float32)
    query0 = np.asarray(query0, dtype=np.float32)
    query1 = np.asarray(query1, dtype=np.float32)
    n_edges = key.shape[0]

    dst = np.asarray(edge_index[1], dtype=np.int64)
    deg = np.bincount(dst, minlength=n_nodes).astype(np.int64)
    n_pad = -(-n_nodes // (P * n_cores)) * (P * n_cores)  # round up to 1024
    deg_pad = np.concatenate([deg, np.zeros(n_pad - n_nodes, dtype=np.int64)])
    nb = n_pad // P
    ng = nb // n_cores

    order = np.argsort(deg_pad, kind="stable")  # node ids, degree-ascending
    degs_o = deg_pad[order]

    blk_max = degs_o.reshape(nb, P).max(axis=1)
    D_eff = np.maximum(blk_max.reshape(ng, n_cores).max(axis=1), 1).astype(np.int64)
    D_eff = (D_eff + D_ROUND - 1) // D_ROUND * D_ROUND
    off = np.concatenate([[0], np.cumsum(P * D_eff)]).astype(np.int64)
    S = int(off[-1])  # slots per core

    pos = np.arange(n_pad)
    block = pos // P
    g_of = block // n_cores
    core_of = block % n_cores
    row = pos % P
    Dg = D_eff[g_of]
    base = off[g_of] + row * Dg

    edge_order = np.argsort(dst, kind="stable")
    starts = np.concatenate([[0], np.cumsum(deg)])

    pp = np.repeat(pos, degs_o)
    cum0 = np.concatenate([[0], np.cumsum(degs_o)])[:-1]
    d_idx = np.arange(n_edges) - np.repeat(cum0, degs_o)
    node_of_pp = order[pp]
    edge_ids = edge_order[starts[node_of_pp] + d_idx]
    slot_global = core_of[pp] * S + base[pp] + d_idx

    kp = np.zeros((n_cores * S, 128), dtype=np.float32)
    kp[slot_global] = key[edge_ids]
    vp = np.zeros((n_cores * S, 96), dtype=np.float32)
    vp[slot_global] = value.reshape(n_edges, 96)[:, PERM_V][edge_ids]
    kp = kp.reshape(n_cores, S, 128).astype(DT_NP)
    vp = vp.reshape(n_cores, S, 96).astype(DT_NP)

    qfull = np.concatenate([query0, query1], axis=-1).reshape(n_nodes, 128)
    q_pad = np.zeros((n_pad, 128), dtype=np.float32)
    q_pad[:n_nodes] = qfull
    q_sorted = q_pad[order].reshape(nb, P, 128)

    pc = (Dg - degs_o).astype(np.float32)
    zero_deg = degs_o == 0
    pc[zero_deg] = (Dg[zero_deg] - 1).astype(np.float32)
    pc_sorted = pc.reshape(nb, P)

    in_maps = []
    for c in range(n_cores):
        # interleaved kv: per (group, node) -> k slots [D,128] then v [D,96]
        parts = []
        for g in range(ng):
            D = int(D_eff[g])
            s0, s1 = int(off[g]), int(off[g + 1])
            kb = kp[c, s0:s1].reshape(P, D * 128)
            vb = vp[c, s0:s1].reshape(P, D * 96)
            parts.append(np.concatenate([kb, vb], axis=1).reshape(-1))
        kv_c = np.concatenate(parts)
        q_c = np.ascontiguousarray(
            q_sorted[c::n_cores].transpose(1, 0, 2).reshape(P, ng * 128)).astype(DT_NP)
        pc_c = np.repeat(np.ascontiguousarray(pc_sorted[c::n_cores].T), H, axis=1)
        in_maps.append({"kv": kv_c, "q": q_c, "pc": pc_c})

    meta = dict(D_eff=D_eff, off=off, S=S, NG=ng, NB=nb, order=order,
                n_nodes=n_nodes, n_pad=n_pad)
    return in_maps, meta


def unshard_output(out_cores, meta):
    """out_cores: list of [128, NG*96] -> [n_nodes, 32, 3]."""
    ng, nb = meta["NG"], meta["NB"]
    n_cores = len(out_cores)
    order, n_nodes, n_pad = meta["order"], meta["n_nodes"], meta["n_pad"]
    out_sorted = np.zeros((nb, P, 96), dtype=np.float32)
    for c in range(n_cores):
        out_sorted[c::n_cores] = (
            out_cores[c].reshape(P, ng, 96).transpose(1, 0, 2))
    out_sorted = out_sorted.reshape(n_pad, 96)[:, PERM_V_INV]
    out_full = np.zeros((n_nodes, 96), dtype=np.float32)
    mask = order < n_nodes
    out_full[order[mask]] = out_sorted[mask]
    return out_full.reshape(n_nodes, 32, 3)


# ---------------------------------------------------------------- bass kernel

def merge_runs(D_eff, cap=MERGE_CAP):
    """[(g0, GM, D), ...]: consecutive equal-D groups fused while GM*D <= cap."""
    runs = []
    g = 0
    ng = len(D_eff)
    while g < ng:
        D = int(D_eff[g])
        gm = 1
        while g + gm < ng and int(D_eff[g + gm]) == D and (gm + 1) * D <= cap:
            gm += 1
        runs.append((g, gm, D))
        g += gm
    return runs


def build(D_eff, S, NG, n_cores=N_CORES):
    D_eff = [int(d) for d in D_eff]
    off = np.concatenate([[0], np.cumsum([P * d for d in D_eff])]).astype(np.int64)

    nc = bacc.Bacc("TRN2", target_bir_lowering=False, debug=False,
                   num_devices=n_cores)
    DT = mybir.dt.bfloat16
    kv = nc.declare_dram_parameter("kv", [S * 224], DT, isOutput=False)
    q = nc.declare_dram_parameter("q", [P, NG * 128], DT, isOutput=False)
    pc = nc.declare_dram_parameter("pc", [P, NG * H], F32, isOutput=False)
    out = nc.declare_dram_parameter("out", [P, NG * 96], F32, isOutput=True)

    mult = mybir.AluOpType.mult
    add = mybir.AluOpType.add
    AX = mybir.AxisListType.X
    runs = merge_runs(D_eff)

    with tile.TileContext(nc) as tc:
        with tc.tile_pool(name="res", bufs=1) as res, \
             tc.tile_pool(name="work3", bufs=3) as work3, \
             tc.tile_pool(name="work", bufs=2) as work, \
             tc.tile_pool(name="small", bufs=2) as small, \
             tc.tile_pool(name="tv", bufs=1) as tvp:
            q_sb = res.tile([P, NG * 128], DT)
            nc.sync.dma_start(q_sb[:], q[:])
            pc_sb = res.tile([P, NG * H], F32)
            nc.sync.dma_start(pc_sb[:], pc[:])
            out_sb = res.tile([P, NG * 96], F32)
            ss_all = res.tile([P, NG * H], F32)

            def emit_value_side(st):
                """v*ewx multiply + d-halvings + reduce for a previous run
                (software pipelining: its ewx is long done, so nothing
                stalls)."""
                g0, GM, D, kvt, ewx = st
                W = 224 * D
                D2, D4 = D // 2, D // 4
                kvg = kvt[:].rearrange("n (g w) -> n g w", g=GM)
                vt = kvg[:, :, 128 * D:]  # [n, g, (d c h)]
                wv = work3.tile([P, GM * D * 96], DT, tag="wv")
                wv3 = wv[:].rearrange("n (g dch) -> n g dch", g=GM)
                ewx3 = ewx[:].rearrange("n (g dch) -> n g dch", g=GM)
                sv = 96 * (D - int(round(D * GP_FRAC_VMUL)))
                if sv < 96 * D:  # GPSIMD slice first so its queue runs ahead
                    nc.gpsimd.tensor_tensor(
                        out=wv3[:, :, sv:], in0=vt[:, :, sv:],
                        in1=ewx3[:, :, sv:], op=mult)
                if sv > 0:
                    nc.vector.tensor_tensor(
                        out=wv3[:, :, :sv], in0=vt[:, :, :sv],
                        in1=ewx3[:, :, :sv], op=mult)
                # two contiguous d-halvings, then a small strided reduce
                tv1 = tvp.tile([P, GM * D2 * 96], DT, tag="tv1")
                wvh = wv[:].rearrange("n (g two dch) -> n g two dch",
                                      g=GM, two=2)
                nc.vector.tensor_tensor(
                    out=tv1[:].rearrange("n (g dch) -> n g dch", g=GM),
                    in0=wvh[:, :, 0], in1=wvh[:, :, 1], op=add)
                tv2 = tvp.tile([P, GM * D4 * 96], DT, tag="tv2")
                tv1h = tv1[:].rearrange("n (g two dch) -> n g two dch",
                                        g=GM, two=2)
                nc.vector.tensor_tensor(
                    out=tv2[:].rearrange("n (g dch) -> n g dch", g=GM),
                    in0=tv1h[:, :, 0], in1=tv1h[:, :, 1], op=add)
                nc.vector.tensor_reduce(
                    out=(out_sb[:, g0 * 96:(g0 + GM) * 96]
                         .rearrange("n (g ch) -> n g ch", g=GM)),
                    in_=tv2[:].rearrange("n (g d ch) -> n g ch d",
                                         g=GM, ch=96),
                    axis=AX, op=add)

            dn_all = res.tile([P, NG * H], F32)
            rs_all = res.tile([P, NG * H], F32)
            rsx = res.tile([P, NG * 96], F32)

            def emit_norm(lo, hi):
                """Normalize + store groups [lo, hi) (out_sb rows final)."""
                nc.vector.tensor_sub(out=dn_all[:, lo * H:hi * H],
                                     in0=ss_all[:, lo * H:hi * H],
                                     in1=pc_sb[:, lo * H:hi * H])
                nc.vector.reciprocal(out=rs_all[:, lo * H:hi * H],
                                     in_=dn_all[:, lo * H:hi * H])
                ngc = hi - lo
                nc.scalar.copy(
                    out=(rsx[:, lo * 96:hi * 96]
                         .rearrange("n (g c h) -> n g c h", g=ngc, c=12)),
                    in_=(rs_all[:, lo * H:hi * H]
                         .rearrange("n (g h) -> n g h", g=ngc)
                         .unsqueeze(2).broadcast_to([P, ngc, 12, H])))
                nc.vector.tensor_tensor(out=out_sb[:, lo * 96:hi * 96],
                                        in0=out_sb[:, lo * 96:hi * 96],
                                        in1=rsx[:, lo * 96:hi * 96], op=mult)
                nc.sync.dma_start(out[:, lo * 96:hi * 96],
                                  out_sb[:, lo * 96:hi * 96])

            split_i = max(1, len(runs) - 3)
            prev = None
            for i, (g0, GM, D) in enumerate(runs):
                W = 224 * D  # kv cols per group
                kvt = work3.tile([P, GM * W], DT, tag="kvt")
                for j in range(GM):
                    s0 = int(off[g0 + j])
                    nc.sync.dma_start(
                        kvt[:, j * W:(j + 1) * W],
                        kv[s0 * 224:(s0 + P * D) * 224]
                        .rearrange("(n w) -> n w", n=P))

                kvg = kvt[:].rearrange("n (g w) -> n g w", g=GM)
                kt = kvg[:, :, :128 * D].rearrange("n g (d f) -> n g d f", f=128)

                # w = k * q  (q broadcast over d: middle dim, inner run 128)
                qb = (q_sb[:, g0 * 128:(g0 + GM) * 128]
                      .rearrange("n (g f) -> n g f", g=GM)
                      .unsqueeze(2).broadcast_to([P, GM, D, 128]))
                w = work.tile([P, GM * D * 128], DT, tag="w")
                w4 = w[:].rearrange("n (g d f) -> n g d f", g=GM, f=128)
                dv = D - int(round(D * GP_FRAC_KMUL))
                if dv < D:  # GPSIMD slice first so its queue runs ahead
                    nc.gpsimd.tensor_tensor(
                        out=w4[:, :, dv:], in0=kt[:, :, dv:], in1=qb[:, :, dv:],
                        op=mult)
                if dv > 0:
                    nc.vector.tensor_tensor(
                        out=w4[:, :, :dv], in0=kt[:, :, :dv], in1=qb[:, :, :dv],
                        op=mult)

                # logits = reduce over contiguous inner k=16 -> [n, (g d h)]
                lg = small.tile([P, GM * D * H], F32, tag="lg")
                nc.vector.tensor_reduce(
                    out=lg[:].rearrange("n (g dh) -> n g dh", g=GM),
                    in_=w[:].rearrange("n (g dh k) -> n g dh k", g=GM, k=16),
                    axis=AX, op=add)

                # ew = exp(scale * logits)  (contiguous [g, d, h]; feeds ewx)
                ew = small.tile([P, GM * D * H], DT, tag="ew")
                nc.scalar.activation(
                    out=ew[:], in_=lg[:],
                    func=mybir.ActivationFunctionType.Exp, scale=SCALE)
                # second exp transposed to [g, h, d]: the denominator reduce
                # becomes contiguous-inner (1.04 vs 2.5 ns/elem strided)
                ew2 = small.tile([P, GM * D * H], DT, tag="ew2")
                nc.scalar.activation(
                    out=ew2[:].rearrange("n (g h d) -> n g d h", g=GM, h=H),
                    in_=lg[:].rearrange("n (g d h) -> n g d h", g=GM, h=H),
                    func=mybir.ActivationFunctionType.Exp, scale=SCALE)

                # ewx: ew expanded over the 12 c-channels (ScalarE; broadcast
                # on a middle dim with contiguous inner run 8)
                ewx = small.tile([P, GM * D * 96], DT, tag="ewx")
                nc.scalar.copy(
                    out=ewx[:].rearrange("n (gd c h) -> n gd c h", c=12, h=H),
                    in_=(ew[:].rearrange("n (gd h) -> n gd h", h=H)
                         .unsqueeze(2).broadcast_to([P, GM * D, 12, H])))

                # deferred value side of the previous run
                if prev is not None:
                    emit_value_side(prev)

                # denominator: contiguous-inner reduce of the transposed exp;
                # emitted late so ScalarE has finished
                nc.vector.tensor_reduce(
                    out=ss_all[:, g0 * H:(g0 + GM) * H],
                    in_=ew2[:].rearrange("n (gh d) -> n gh d", d=D),
                    axis=AX, op=add)

                if i == split_i:
                    emit_norm(0, runs[i][0])

                prev = (g0, GM, D, kvt, ewx)

            emit_value_side(prev)
            emit_norm(runs[split_i][0], NG)

    nc.compile()
    return nc


# ---------------------------------------------------------------- entry point

LAST_RESULT = None  # BassKernelResults of the most recent run (for test harness)


def kernel(value, key, query0, query1, edge_index):
    global LAST_RESULT
    import os
    in_maps, meta = prepare(value, key, query0, query1, edge_index)
    nc = build(meta["D_eff"], meta["S"], meta["NG"])
    res = run_bass_kernel_spmd(nc, in_maps, list(range(N_CORES)),
                               tmpdir=os.environ.get("BASS_SPMD_TMPDIR"))
    LAST_RESULT = res
    out_cores = [res.results[c]["out"] for c in range(N_CORES)]
    return unshard_output(out_cores, meta)


# revision 29
# speedup vs baseline: 1.1730x; 1.1730x over previous
"""Trainium2 Bass kernel for nn_AttentionSE3 (graph attention message passing).

Strategy (edge/graph parallel, fully host-prepped ELL layout):
- Attention is a segment softmax over incoming edges of each dst node.  Logits are
  dot(k_edge, q_dst)/sqrt(128) with k,q ~ N(0,1): |logit| <~ 2, so the max-subtraction
  is dropped (softmax is shift-invariant; exp() never overflows here) and
  out[n] = sum_e exp(logit_e) * v_e / sum_e exp(logit_e).
- Host sorts nodes by in-degree, packs them into 128-node blocks, and pads each
  block's per-node edge lists to the block max degree D (rounded to a multiple
  of 4; degree sorting keeps padding small).  Blocks are dealt round-robin to
  the 8 cores; per-group capacity is the max over the 8 cores so EVERY core
  runs the same static program (no collectives: no node's edges span cores).
- Host packs ONE interleaved kv array: per (group, node-row) the D key rows
  [D,128] then the D value rows [D,96] (value columns permuted to [c12,h8]).
  One DMA per group loads both.
- Device program per group-run (consecutive equal-D groups merge into one set
  of wide ops), software-pipelined so no engine's in-order queue waits on
  same-iteration cross-engine results:
  iteration i emits:  k*q multiply(i) (q broadcast over d; a SMALL slice on
  GPSIMD -- heavy concurrent GPSIMD work poisons VectorE's SBUF perf modes);
  VectorE tensor_reduce over contiguous inner k=16 -> logits(i); ScalarE
  exp(i) and the c-expansion ewx(i); then the DEFERRED value side of run i-1
  (multiply, two contiguous d-halvings, strided reduce); then the
  denominator reduce(i).  Padded slots contribute exp(0)=1; a per-node pad
  count is subtracted (exact).  Output accumulates in SBUF; one wide
  normalization pass + one store at the end.
"""

import numpy as np

import concourse.bacc as bacc
import concourse.mybir as mybir
from concourse import tile
from concourse.bass_utils import run_bass_kernel_spmd

try:
    import ml_dtypes
    BF16_NP = np.dtype(ml_dtypes.bfloat16)
except ImportError:  # pragma: no cover
    BF16_NP = None

N_NODES = 50000
H = 8
P = 128  # nodes per block
N_CORES = 8
SCALE = float(1.0 / np.sqrt(128.0))
F32 = mybir.dt.float32
DT_NP = BF16_NP

# engine split knobs (fraction to GPSIMD; VectorE takes the rest).  Keep these
# SMALL: concurrent GPSIMD traffic degrades VectorE's SBUF perf modes.
GP_FRAC_KMUL = 0.22   # of the k*q multiply d-range
GP_FRAC_VMUL = 0.12   # of the v*ewx multiply d-range
D_ROUND = 4           # capacities rounded up to this multiple (2 halvings)
MERGE_CAP = 48        # max merged capacity per run (SBUF-bounded)

# value columns permuted from [h(8), cx(12)] to [cx(12), h(8)] to match the
# ewx expansion layout (c outer, h inner)
PERM_V = np.arange(96).reshape(8, 12).T.reshape(-1)  # new cx*8+h -> old h*12+cx
PERM_V_INV = np.argsort(PERM_V)


# ---------------------------------------------------------------- host prep

def prepare(value, key, query0, query1, edge_index, n_nodes=N_NODES, n_cores=N_CORES):
    """Build per-core padded ELL shards.  Returns (in_maps, meta)."""
    value = np.asarray(value, dtype=np.float32)
    key = np.asarray(key, dtype=np.float32)
    query0 = np.asarray(query0, dtype=np.float32)
    query1 = np.asarray(query1, dtype=np.float32)
    n_edges = key.shape[0]

    dst = np.asarray(edge_index[1], dtype=np.int64)
    deg = np.bincount(dst, minlength=n_nodes).astype(np.int64)
    n_pad = -(-n_nodes // (P * n_cores)) * (P * n_cores)  # round up to 1024
    deg_pad = np.concatenate([deg, np.zeros(n_pad - n_nodes, dtype=np.int64)])
    nb = n_pad // P
    ng = nb // n_cores

    order = np.argsort(deg_pad, kind="stable")  # node ids, degree-ascending
    degs_o = deg_pad[order]

    blk_max = degs_o.reshape(nb, P).max(axis=1)
    D_eff = np.maximum(blk_max.reshape(ng, n_cores).max(axis=1), 1).astype(np.int64)
    D_eff = (D_eff + D_ROUND - 1) // D_ROUND * D_ROUND
    off = np.concatenate([[0], np.cumsum(P * D_eff)]).astype(np.int64)
    S = int(off[-1])  # slots per core

    pos = np.arange(n_pad)
    block = pos // P
    g_of = block // n_cores
    core_of = block % n_cores
    row = pos % P
    Dg = D_eff[g_of]
    base = off[g_of] + row * Dg

    edge_order = np.argsort(dst, kind="stable")
    starts = np.concatenate([[0], np.cumsum(deg)])

    pp = np.repeat(pos, degs_o)
    cum0 = np.concatenate([[0], np.cumsum(degs_o)])[:-1]
    d_idx = np.arange(n_edges) - np.repeat(cum0, degs_o)
    node_of_pp = order[pp]
    edge_ids = edge_order[starts[node_of_pp] + d_idx]
    slot_global = core_of[pp] * S + base[pp] + d_idx

    kp = np.zeros((n_cores * S, 128), dtype=np.float32)
    kp[slot_global] = key[edge_ids]
    vp = np.zeros((n_cores * S, 96), dtype=np.float32)
    vp[slot_global] = value.reshape(n_edges, 96)[:, PERM_V][edge_ids]
    kp = kp.reshape(n_cores, S, 128).astype(DT_NP)
    vp = vp.reshape(n_cores, S, 96).astype(DT_NP)

    qfull = np.concatenate([query0, query1], axis=-1).reshape(n_nodes, 128)
    q_pad = np.zeros((n_pad, 128), dtype=np.float32)
    q_pad[:n_nodes] = qfull
    q_sorted = q_pad[order].reshape(nb, P, 128)

    pc = (Dg - degs_o).astype(np.float32)
    zero_deg = degs_o == 0
    pc[zero_deg] = (Dg[zero_deg] - 1).astype(np.float32)
    pc_sorted = pc.reshape(nb, P)

    in_maps = []
    for c in range(n_cores):
        # interleaved kv: per (group, node) -> k slots [D,128] then v [D,96]
        parts = []
        for g in range(ng):
            D = int(D_eff[g])
            s0, s1 = int(off[g]), int(off[g + 1])
            kb = kp[c, s0:s1].reshape(P, D * 128)
            vb = vp[c, s0:s1].reshape(P, D * 96)
            parts.append(np.concatenate([kb, vb], axis=1).reshape(-1))
        kv_c = np.concatenate(parts)
        q_c = np.ascontiguousarray(
            q_sorted[c::n_cores].transpose(1, 0, 2).reshape(P, ng * 128)).astype(DT_NP)
        pc_c = np.repeat(np.ascontiguousarray(pc_sorted[c::n_cores].T), H, axis=1)
        in_maps.append({"kv": kv_c, "q": q_c, "pc": pc_c})

    meta = dict(D_eff=D_eff, off=off, S=S, NG=ng, NB=nb, order=order,
                n_nodes=n_nodes, n_pad=n_pad)
    return in_maps, meta


def unshard_output(out_cores, meta):
    """out_cores: list of [128, NG*96] -> [n_nodes, 32, 3]."""
    ng, nb = meta["NG"], meta["NB"]
    n_cores = len(out_cores)
    order, n_nodes, n_pad = meta["order"], meta["n_nodes"], meta["n_pad"]
    out_sorted = np.zeros((nb, P, 96), dtype=np.float32)
    for c in range(n_cores):
        out_sorted[c::n_cores] = (
            out_cores[c].reshape(P, ng, 96).transpose(1, 0, 2))
    out_sorted = out_sorted.reshape(n_pad, 96)[:, PERM_V_INV]
    out_full = np.zeros((n_nodes, 96), dtype=np.float32)
    mask = order < n_nodes
    out_full[order[mask]] = out_sorted[mask]
    return out_full.reshape(n_nodes, 32, 3)


# ---------------------------------------------------------------- bass kernel

def merge_runs(D_eff, cap=MERGE_CAP):
    """[(g0, GM, D), ...]: consecutive equal-D groups fused while GM*D <= cap."""
    runs = []
    g = 0
    ng = len(D_eff)
    while g < ng:
        D = int(D_eff[g])
        gm = 1
        while g + gm < ng and int(D_eff[g + gm]) == D and (gm + 1) * D <= cap:
            gm += 1
        runs.append((g, gm, D))
        g += gm
    return runs


def build(D_eff, S, NG, n_cores=N_CORES):
    D_eff = [int(d) for d in D_eff]
    off = np.concatenate([[0], np.cumsum([P * d for d in D_eff])]).astype(np.int64)

    nc = bacc.Bacc("TRN2", target_bir_lowering=False, debug=False,
                   num_devices=n_cores)
    DT = mybir.dt.bfloat16
    kv = nc.declare_dram_parameter("kv", [S * 224], DT, isOutput=False)
    q = nc.declare_dram_parameter("q", [P, NG * 128], DT, isOutput=False)
    pc = nc.declare_dram_parameter("pc", [P, NG * H], F32, isOutput=False)
    out = nc.declare_dram_parameter("out", [P, NG * 96], F32, isOutput=True)

    mult = mybir.AluOpType.mult
    add = mybir.AluOpType.add
    AX = mybir.AxisListType.X
    runs = merge_runs(D_eff)

    with tile.TileContext(nc) as tc:
        with tc.tile_pool(name="res", bufs=1) as res, \
             tc.tile_pool(name="work3", bufs=3) as work3, \
             tc.tile_pool(name="work", bufs=2) as work, \
             tc.tile_pool(name="small", bufs=2) as small, \
             tc.tile_pool(name="tv", bufs=1) as tvp:
            q_sb = res.tile([P, NG * 128], DT)
            nc.sync.dma_start(q_sb[:], q[:])
            pc_sb = res.tile([P, NG * H], F32)
            nc.sync.dma_start(pc_sb[:], pc[:])
            out_sb = res.tile([P, NG * 96], F32)
            ss_all = res.tile([P, NG * H], F32)

            def emit_value_side(st):
                """v*ewx multiply + d-halvings + reduce for a previous run
                (software pipelining: its ewx is long done, so nothing
                stalls)."""
                g0, GM, D, kvt, ewx = st
                W = 224 * D
                D2, D4 = D // 2, D // 4
                kvg = kvt[:].rearrange("n (g w) -> n g w", g=GM)
                vt = kvg[:, :, 128 * D:]  # [n, g, (d c h)]
                wv = work3.tile([P, GM * D * 96], DT, tag="wv")
                wv3 = wv[:].rearrange("n (g dch) -> n g dch", g=GM)
                ewx3 = ewx[:].rearrange("n (g dch) -> n g dch", g=GM)
                sv = 96 * (D - int(round(D * GP_FRAC_VMUL)))
                if sv < 96 * D:  # GPSIMD slice first so its queue runs ahead
                    nc.gpsimd.tensor_tensor(
                        out=wv3[:, :, sv:], in0=vt[:, :, sv:],
                        in1=ewx3[:, :, sv:], op=mult)
                if sv > 0:
                    nc.vector.tensor_tensor(
                        out=wv3[:, :, :sv], in0=vt[:, :, :sv],
                        in1=ewx3[:, :, :sv], op=mult)
                # two contiguous d-halvings, then a small strided reduce
                tv1 = tvp.tile([P, GM * D2 * 96], DT, tag="tv1")
                wvh = wv[:].rearrange("n (g two dch) -> n g two dch",
                                      g=GM, two=2)
                nc.vector.tensor_tensor(
                    out=tv1[:].rearrange("n (g dch) -> n g dch", g=GM),
                    in0=wvh[:, :, 0], in1=wvh[:, :, 1], op=add)
                tv2 = tvp.tile([P, GM * D4 * 96], DT, tag="tv2")
                tv1h = tv1[:].rearrange("n (g two dch) -> n g two dch",
                                        g=GM, two=2)
                nc.vector.tensor_tensor(
                    out=tv2[:].rearrange("n (g dch) -> n g dch", g=GM),
                    in0=tv1h[:, :, 0], in1=tv1h[:, :, 1], op=add)
                nc.vector.tensor_reduce(
                    out=(out_sb[:, g0 * 96:(g0 + GM) * 96]
                         .rearrange("n (g ch) -> n g ch", g=GM)),
                    in_=tv2[:].rearrange("n (g d ch) -> n g ch d",
                                         g=GM, ch=96),
                    axis=AX, op=add)

            prev = None
            for (g0, GM, D) in runs:
                W = 224 * D  # kv cols per group
                kvt = work3.tile([P, GM * W], DT, tag="kvt")
                for j in range(GM):
                    s0 = int(off[g0 + j])
                    nc.sync.dma_start(
                        kvt[:, j * W:(j + 1) * W],
                        kv[s0 * 224:(s0 + P * D) * 224]
                        .rearrange("(n w) -> n w", n=P))

                kvg = kvt[:].rearrange("n (g w) -> n g w", g=GM)
                kt = kvg[:, :, :128 * D].rearrange("n g (d f) -> n g d f", f=128)

                # w = k * q  (q broadcast over d: middle dim, inner run 128)
                qb = (q_sb[:, g0 * 128:(g0 + GM) * 128]
                      .rearrange("n (g f) -> n g f", g=GM)
                      .unsqueeze(2).broadcast_to([P, GM, D, 128]))
                w = work.tile([P, GM * D * 128], DT, tag="w")
                w4 = w[:].rearrange("n (g d f) -> n g d f", g=GM, f=128)
                dv = D - int(round(D * GP_FRAC_KMUL))
                if dv < D:  # GPSIMD slice first so its queue runs ahead
                    nc.gpsimd.tensor_tensor(
                        out=w4[:, :, dv:], in0=kt[:, :, dv:], in1=qb[:, :, dv:],
                        op=mult)
                if dv > 0:
                    nc.vector.tensor_tensor(
                        out=w4[:, :, :dv], in0=kt[:, :, :dv], in1=qb[:, :, :dv],
                        op=mult)

                # logits = reduce over contiguous inner k=16 -> [n, (g d h)]
                lg = small.tile([P, GM * D * H], F32, tag="lg")
                nc.vector.tensor_reduce(
                    out=lg[:].rearrange("n (g dh) -> n g dh", g=GM),
                    in_=w[:].rearrange("n (g dh k) -> n g dh k", g=GM, k=16),
                    axis=AX, op=add)

                # ew = exp(scale * logits)  (contiguous [g, d, h])
                ew = small.tile([P, GM * D * H], DT, tag="ew")
                nc.scalar.activation(
                    out=ew[:], in_=lg[:],
                    func=mybir.ActivationFunctionType.Exp, scale=SCALE)

                # ewx: ew expanded over the 12 c-channels (ScalarE; broadcast
                # on a middle dim with contiguous inner run 8)
                ewx = small.tile([P, GM * D * 96], DT, tag="ewx")
                nc.scalar.copy(
                    out=ewx[:].rearrange("n (gd c h) -> n gd c h", c=12, h=H),
                    in_=(ew[:].rearrange("n (gd h) -> n gd h", h=H)
                         .unsqueeze(2).broadcast_to([P, GM * D, 12, H])))

                # deferred value side of the previous run
                if prev is not None:
                    emit_value_side(prev)

                # denominator: sum over d (strided middle, F=8/edge); emitted
                # late so ScalarE's exp has finished
                nc.vector.tensor_reduce(
                    out=(ss_all[:, g0 * H:(g0 + GM) * H]
                         .rearrange("n (g h) -> n g h", g=GM)),
                    in_=(ew[:].rearrange("n (g d h) -> n g h d", g=GM, h=H)),
                    axis=AX, op=add)

                prev = (g0, GM, D, kvt, ewx)

            emit_value_side(prev)

            # one wide deferred normalization pass
            dn_all = res.tile([P, NG * H], F32)
            nc.vector.tensor_sub(out=dn_all[:], in0=ss_all[:], in1=pc_sb[:])
            rs_all = res.tile([P, NG * H], F32)
            nc.vector.reciprocal(out=rs_all[:], in_=dn_all[:])
            rsx = res.tile([P, NG * 96], F32)
            nc.scalar.copy(
                out=rsx[:].rearrange("n (g c h) -> n g c h", g=NG, c=12),
                in_=(rs_all[:].rearrange("n (g h) -> n g h", g=NG)
                     .unsqueeze(2).broadcast_to([P, NG, 12, H])))
            nc.vector.tensor_tensor(out=out_sb[:], in0=out_sb[:], in1=rsx[:],
                                    op=mult)

            nc.sync.dma_start(out[:], out_sb[:])

    nc.compile()
    return nc


# ---------------------------------------------------------------- entry point

LAST_RESULT = None  # BassKernelResults of the most recent run (for test harness)


def kernel(value, key, query0, query1, edge_index):
    global LAST_RESULT
    import os
    in_maps, meta = prepare(value, key, query0, query1, edge_index)
    nc = build(meta["D_eff"], meta["S"], meta["NG"])
    res = run_bass_kernel_spmd(nc, in_maps, list(range(N_CORES)),
                               tmpdir=os.environ.get("BASS_SPMD_TMPDIR"))
    LAST_RESULT = res
    out_cores = [res.results[c]["out"] for c in range(N_CORES)]
    return unshard_output(out_cores, meta)
